# revision 1
# baseline (speedup 1.0000x reference)
"""Trainium2 Bass kernel for EnhancedGlobalGNN (B=8, N=1024, HID=64, H=4, D=16).

Sharding: batch dim B=8 across 8 cores (pure data parallel, params replicated,
no collectives). Each core computes one batch element end-to-end on chip.

Layout strategy per core:
  - adj is DMA'd once (4MB), cast fp16, transposed on-chip via the DMA xbar to
    adjT[c, n] = adj[n, c].  adjT is simultaneously: the rhs for the input
    projection, the source of deg (ones-matmul colsum of |adjT|), and the
    attention mask maskT[m, n] = (adj[n, m] > 0).
  - activations x live in "natural" tiles (128 nodes x 64 feat) for LN/gelu and
    are PE-transposed to xT (64, 1024) fp16 per round for projections.
  - GAT scores: e[m, n] layout (partition = source node m).  u = s_n + t_m via
    one tensor_scalar (s broadcast along partitions by DMA, t as per-partition
    column from PE).  lrelu = max(u, 0.2u) on DVE, exp on ACT (fp16),
    mask multiply on DVE.  attn@h via PE with a 17th ones-column on h giving
    the softmax denominator for free.
"""

import numpy as np

import concourse.bass as bass
import concourse.mybir as mybir
import concourse.tile as tile
from concourse import bacc
from concourse.bass import ds, ts
from concourse.masks import make_identity

F32 = mybir.dt.float32
F16 = mybir.dt.float16
I32 = mybir.dt.int32
AF = mybir.ActivationFunctionType
ALU = mybir.AluOpType

B, N, H, D, HID, NCOMM, OUT = 8, 1024, 4, 16, 64, 20, 128
P = 128
NT = N // P  # 8 node tiles
NB = N // 512  # 2 psum banks per N row
EPS = 1e-5
HA = 17  # per-head aug width (16 dims + ones col)


def rep_ap(ap2d, reps):
    """View a (P, F) AP as (P, reps, F) with stride-0 repeat along the middle."""
    return bass.AP(
        tensor=ap2d.tensor,
        offset=ap2d.offset,
        ap=[ap2d.ap[0], [0, reps]] + list(ap2d.ap[1:]),
    )


def bcast_ap(ap_row, nparts):
    """View a (1, F) AP as (nparts, F) with stride-0 partition broadcast."""
    return bass.AP(
        tensor=ap_row.tensor,
        offset=ap_row.offset,
        ap=[[0, nparts]] + list(ap_row.ap[1:]),
    )


def build(nc: bass.Bass):
    # ---------------- DRAM I/O (per-core shard: one batch element) ----------
    adj = nc.dram_tensor("adj", [N, N], F32, kind="ExternalInput")
    coords = nc.dram_tensor("coords", [N, 3], F32, kind="ExternalInput")
    comm = nc.dram_tensor("comm", [1, N], I32, kind="ExternalInput")
    in_w = nc.dram_tensor("in_w", [HID, N + 4], F32, kind="ExternalInput")
    in_b = nc.dram_tensor("in_b", [1, HID], F32, kind="ExternalInput")
    in_ln_g = nc.dram_tensor("in_ln_g", [1, HID], F32, kind="ExternalInput")
    in_ln_b = nc.dram_tensor("in_ln_b", [1, HID], F32, kind="ExternalInput")
    comm_table = nc.dram_tensor("comm_table", [NCOMM + 1, HID], F32, kind="ExternalInput")
    gate_w = nc.dram_tensor("gate_w", [HID, 2 * HID], F32, kind="ExternalInput")
    gate_b = nc.dram_tensor("gate_b", [1, HID], F32, kind="ExternalInput")
    gat_W = nc.dram_tensor("gat_W", [3, HID, HID], F32, kind="ExternalInput")
    gat_a = nc.dram_tensor("gat_a", [3, H, 2 * D], F32, kind="ExternalInput")
    gat_ln_g = nc.dram_tensor("gat_ln_g", [3, HID], F32, kind="ExternalInput")
    gat_ln_b = nc.dram_tensor("gat_ln_b", [3, HID], F32, kind="ExternalInput")
    pool_w = nc.dram_tensor("pool_w", [1, HID], F32, kind="ExternalInput")
    pool_b = nc.dram_tensor("pool_b", [1, 1], F32, kind="ExternalInput")
    out_w = nc.dram_tensor("out_w", [OUT, 3 * HID], F32, kind="ExternalInput")
    out_b = nc.dram_tensor("out_b", [1, OUT], F32, kind="ExternalInput")
    out_ln_g = nc.dram_tensor("out_ln_g", [1, OUT], F32, kind="ExternalInput")
    out_ln_b = nc.dram_tensor("out_ln_b", [1, OUT], F32, kind="ExternalInput")
    out_y = nc.dram_tensor("out", [1, OUT], F32, kind="ExternalOutput")

    with tile.TileContext(nc) as tc:
        _body(nc, tc, locals())
    nc.compile()
    return nc


def _body(nc, tc, t):
    adj, coords, comm = t["adj"], t["coords"], t["comm"]
    sb = tc.alloc_tile_pool(name="sb", bufs=1)          # persistent sbuf
    sb2 = tc.alloc_tile_pool(name="sb2", bufs=2)        # transient double-buffered
    big = tc.alloc_tile_pool(name="big", bufs=2)        # e-pipeline tiles
    ps = tc.alloc_tile_pool(name="ps", bufs=2, space="PSUM")
    psb = tc.alloc_tile_pool(name="psb", bufs=1, space="PSUM")
    ps_o = tc.alloc_tile_pool(name="ps_o", bufs=1, space="PSUM")
    padj = tc.alloc_tile_pool(name="padj", bufs=1)      # adj staging, freed post phase1

    # ---------------- setup: identities, params, transposed weights --------
    ident32 = sb.tile([P, P], F32)
    make_identity(nc, ident32)
    ident16 = sb.tile([P, P], F16)
    make_identity(nc, ident16)

    def dma(out_ap, in_ap):
        nc.sync.dma_start(out=out_ap, in_=in_ap)

    # small params straight to SBUF (fp32); in_w staging freed after phase 1
    in_w_sb = padj.tile([HID, N + 4], F32)
    dma(in_w_sb, t["in_w"][:, :])
    in_b_sb = sb.tile([1, HID], F32)
    dma(in_b_sb, t["in_b"][:, :])
    gate_w_sb = sb.tile([HID, 2 * HID], F32)
    dma(gate_w_sb, t["gate_w"][:, :])
    gate_b_sb = sb.tile([1, HID], F32)
    dma(gate_b_sb, t["gate_b"][:, :])
    ct_sb = sb.tile([NCOMM + 1, HID], F32)
    dma(ct_sb, t["comm_table"][:, :])
    gat_W_sb = sb.tile([HID, 3, HID], F32)
    for l in range(3):
        dma(gat_W_sb[:, l, :], t["gat_W"][l, :, :])
    gat_a_sb = sb.tile([H, 3, 2 * D], F32)
    for l in range(3):
        dma(gat_a_sb[:, l, :], t["gat_a"][l, :, :])
    pool_w_sb = sb.tile([1, HID], F32)
    dma(pool_w_sb, t["pool_w"][:, :])
    pool_b_sb = sb.tile([1, 1], F32)
    dma(pool_b_sb, t["pool_b"][:, :])
    out_w_sb = sb.tile([OUT, 3 * HID], F32)
    dma(out_w_sb, t["out_w"][:, :])
    out_b_sb = sb.tile([1, OUT], F32)
    dma(out_b_sb, t["out_b"][:, :])
    out_ln_g_sb = sb.tile([1, OUT], F32)
    dma(out_ln_g_sb, t["out_ln_g"][:, :])
    out_ln_b_sb = sb.tile([1, OUT], F32)
    dma(out_ln_b_sb, t["out_ln_b"][:, :])

    # per-feature LN params broadcast to 128 partitions (DMA stride-0 from HBM)
    _bc_n = [0]

    def hbm_bcast(dram_row, width):
        _bc_n[0] += 1
        tl = sb.tile([P, width], F32, tag=f"bc{_bc_n[0]}")
        nc.gpsimd.dma_start(out=tl, in_=bcast_ap(dram_row, P))
        return tl

    in_ln_g_bc = hbm_bcast(t["in_ln_g"][:, :], HID)
    in_ln_b_bc = hbm_bcast(t["in_ln_b"][:, :], HID)
    gat_g_bc = [hbm_bcast(t["gat_ln_g"][l : l + 1, :], HID) for l in range(3)]
    gat_b_bc = [hbm_bcast(t["gat_ln_b"][l : l + 1, :], HID) for l in range(3)]

    eps_col = sb.tile([P, 1], F32)
    nc.vector.memset(eps_col, EPS)
    ones_row16 = sb.tile([1, N], F16)
    nc.vector.memset(ones_row16, 1.0)
    ones_col16 = sb.tile([P, 1], F16)
    nc.vector.memset(ones_col16, 1.0)
    ones11_16 = sb.tile([1, 1], F16)
    nc.vector.memset(ones11_16, 1.0)

    # transpose helper: (p, f) fp32/fp16 SBUF -> fp(out) SBUF
    def pe_T(src_ap, p_sz, f_sz, out_ap, out_via=None):
        ident = ident32 if src_ap.dtype == F32 else ident16
        pt = ps.tile([f_sz, p_sz], src_ap.dtype, tag="b1")
        nc.tensor.transpose(pt[:, :], src_ap, ident[:p_sz, :p_sz])
        (out_via or nc.scalar.copy)(out_ap, pt[:, :])

    # transposed weights (fp16)
    in_wT16 = sb.tile([P, NT, HID], F16)  # [c%128, chunk, hid]
    for c in range(NT):
        pe_T(in_w_sb[:, ts(c, P)], HID, P, in_wT16[:, c, :])
    deg_w16 = sb.tile([1, HID], F16)   # in_w col 1024 (deg feature weights)
    pe_T(in_w_sb[:, N : N + 1], HID, 1, deg_w16[:, :])
    coords_w16 = sb.tile([3, HID], F16)  # in_w cols 1025..1027
    pe_T(in_w_sb[:, N + 1 : N + 4], HID, 3, coords_w16[:, :])
    in_b16 = sb.tile([1, HID], F16)
    nc.vector.tensor_copy(in_b16[:, :], in_b_sb[:, :])
    gate_wT16 = sb.tile([2 * HID, HID], F16)
    pe_T(gate_w_sb[:, :], HID, 2 * HID, gate_wT16[:, :])
    # W^T with heads padded to 32-row quads: col h*32+d holds W.T[:, h*16+d]
    gat_WTpad16 = sb.tile([HID, 3, P], F16)
    nc.vector.memset(gat_WTpad16.rearrange("p a b -> p (a b)"), 0.0)
    for l in range(3):
        ptw = ps.tile([HID, HID], F32, tag="b1")
        nc.tensor.transpose(ptw[:, :], gat_W_sb[:, l, :], ident32[:HID, :HID])
        nc.scalar.copy(
            bass.AP(tensor=gat_WTpad16.tensor,
                    offset=gat_WTpad16[:, l, :].offset,
                    ap=[gat_WTpad16.ap[0], [32, H], [1, D]]),
            ptw[:, :].rearrange("p (h d) -> p h d", h=H),
        )
    # a vectors, head h at partition h*32: col 0 = a_src, col 1 = a_dst
    aPad2 = sb.tile([P, 3, 2], F16)
    nc.vector.memset(aPad2.rearrange("p a b -> p (a b)"), 0.0)
    for l in range(3):
        for half in range(2):
            pta = ps.tile([D, H], F32, tag="b1")
            nc.tensor.transpose(pta[:, :], gat_a_sb[:, l, ts(half, D)],
                                ident32[:H, :H])
            for h in range(H):
                nc.vector.tensor_copy(
                    aPad2[h * 32 : h * 32 + D, l, half : half + 1],
                    pta[0:D, h : h + 1])
    out_wT16_a = sb.tile([P, OUT], F16)
    pe_T(out_w_sb[:, 0:P], OUT, P, out_wT16_a[:, :])
    out_wT16_b = sb.tile([HID, OUT], F16)
    pe_T(out_w_sb[:, P : 3 * HID], OUT, HID, out_wT16_b[:, :])
    pool_wT16 = sb.tile([HID, 1], F16)
    pe_T(pool_w_sb[:, :], 1, HID, pool_wT16[:, :])
    ct16 = sb.tile([NCOMM + 1, HID], F16)
    nc.vector.tensor_copy(ct16, ct_sb)
    gate_b16 = sb.tile([1, HID], F16)
    nc.vector.tensor_copy(gate_b16, gate_b_sb)
    out_b16 = sb.tile([1, OUT], F16)
    nc.vector.tensor_copy(out_b16, out_b_sb)

    # ---------------- phase 1: adj load, transpose, mask/deg, input proj ---
    adjT16 = padj.tile([P, NT, N], F16)  # [c%128, chunk, n]
    for j in range(NT):
        a32 = padj.tile([P, N], F32, tag=f"adj32_{j % 2}")
        dma(a32[:, :], adj[ts(j, P), :])
        a16 = padj.tile([P, N], F16, tag=f"adj16_{j % 2}")
        nc.vector.tensor_copy(a16[:, :], a32[:, :])
        # transpose (128, 1024) -> writes column block j of every chunk
        nc.sync.dma_start(
            out=adjT16[:, :, ts(j, P)], in_=a16[:, :], transpose=True
        )
    absT16 = padj.tile([P, NT, N], F16)
    nc.scalar.activation(
        absT16.rearrange("p a b -> p (a b)"),
        adjT16.rearrange("p a b -> p (a b)"), AF.Abs,
    )
    maskT16 = sb.tile([P, NT, N], F16)
    nc.vector.tensor_scalar(
        out=maskT16.rearrange("p a b -> p (a b)"),
        in0=adjT16.rearrange("p a b -> p (a b)"),
        scalar1=0.0, scalar2=None, op0=ALU.is_gt,
    )

    # deg_row = ones^T @ |adjT|  -> (1, N)
    deg_ps = psb.tile([1, N], F32, tag="b2")
    for c in range(NT):
        for b in range(NB):
            nc.tensor.matmul(
                deg_ps[:, ts(b, 512)], ones_col16[:, :], absT16[:, c, ts(b, 512)],
                start=(c == 0), stop=(c == NT - 1),
            )
    deg_row16 = sb.tile([1, N], F16)
    nc.scalar.copy(deg_row16[:, :], deg_ps[:, :])
    coordsT16 = sb.tile([3, N], F16)
    crd32 = sb.tile([P, NT, 3], F32)
    dma(crd32[:, :, :], coords.rearrange("(a p) b -> p a b", p=P))
    for j in range(NT):
        pe_T(crd32[:, j, :], P, 3, coordsT16[:, ts(j, P)])

    # xpre = [adj | deg | coords] @ in_w.T + in_b  per node tile, then LN+gelu
    x_nat = sb.tile([P, NT, HID], F32)

    def layer_norm_tile(z_ap, out_ap, g_bc, b_bc, gelu, j):
        stats = sb2.tile([P, 6], F32, tag="lnstats")
        mv = sb2.tile([P, 2], F32, tag="lnmv")
        nc.vector.bn_stats(stats[:, :], z_ap)
        nc.vector.bn_aggr(mv[:, :], stats[:, :])
        sd = sb2.tile([P, 1], F32, tag="lnsd")
        nc.scalar.activation(sd[:, :], mv[:, 1:2], AF.Sqrt, bias=eps_col[:, :], scale=1.0)
        rstd = sb2.tile([P, 1], F32, tag="lnrstd")
        nc.vector.reciprocal(rstd[:, :], sd[:, :])
        xn = sb2.tile([P, HID], F32, tag="lnxn")
        nc.vector.tensor_scalar(
            out=xn[:, :], in0=z_ap, scalar1=mv[:, 0:1], scalar2=rstd[:, :],
            op0=ALU.subtract, op1=ALU.mult,
        )
        xg = sb2.tile([P, HID], F32, tag="lnxg")
        nc.vector.tensor_mul(xg[:, :], xn[:, :], g_bc[:, :])
        nc.vector.tensor_add(xg[:, :], xg[:, :], b_bc[:, :])
        if gelu:
            nc.scalar.activation(out_ap, xg[:, :], AF.Gelu)
        else:
            nc.vector.tensor_copy(out_ap, xg[:, :])

    for j in range(NT):
        xp = ps.tile([P, HID], F32, tag="b1")
        for c in range(NT):
            nc.tensor.matmul(
                xp[:, :], adjT16[:, c, ts(j, P)], in_wT16[:, c, :],
                start=(c == 0), stop=False,
            )
        nc.tensor.matmul(xp[:, :], deg_row16[:, ts(j, P)], deg_w16[:, :],
                         start=False, stop=False)
        nc.tensor.matmul(xp[:, :], coordsT16[:, ts(j, P)], coords_w16[:, :],
                         start=False, stop=False)
        nc.tensor.matmul(xp[:, :], ones_row16[:, ts(j, P)], in_b16[:, :],
                         start=False, stop=True)
        layer_norm_tile(xp[:, :], x_nat[:, j, :], in_ln_g_bc, in_ln_b_bc, True, j)

    padj.release()

    # ---------------- phase 1.5: community gating --------------------------
    comm_f32 = sb.tile([1, N], F32)
    nc.gpsimd.dma_start(out=comm_f32[:, :], in_=comm[:, :])  # casting DMA i32->f32
    comm_bc = sb.tile([NCOMM + 1, N], F32)
    nc.gpsimd.partition_broadcast(comm_bc[:, :], comm_f32[:, :])
    iota_i = sb.tile([NCOMM + 1, 1], I32)
    nc.gpsimd.iota(iota_i[:, :], pattern=[[0, 1]], base=0, channel_multiplier=1)
    iota_f = sb.tile([NCOMM + 1, 1], F32)
    nc.vector.tensor_copy(iota_f, iota_i)
    onehot16 = sb.tile([NCOMM + 1, N], F16)
    nc.vector.tensor_scalar(out=onehot16[:, :], in0=comm_bc[:, :],
                            scalar1=iota_f[:, :], scalar2=None, op0=ALU.is_equal)

    # xT/ceT stacked (128 feat, N) fp16 for the gate matmul
    xce16 = sb.tile([2 * HID, N], F16)
    for j in range(NT):
        pe_T(x_nat[:, j, :], P, HID, xce16[0:HID, ts(j, P)])
    ce_psT = psb.tile([HID, N], F32, tag="b2")
    for b in range(NB):
        nc.tensor.matmul(ce_psT[:, ts(b, 512)], ct16[:, :], onehot16[:, ts(b, 512)],
                         start=True, stop=True)
    nc.scalar.copy(xce16[HID : 2 * HID, :], ce_psT[:, :])

    x_nat2 = sb.tile([P, NT, HID], F32)
    for j in range(NT):
        gp = ps.tile([P, HID], F32, tag="b1")
        nc.tensor.matmul(gp[:, :], xce16[:, ts(j, P)], gate_wT16[:, :],
                         start=True, stop=False)
        nc.tensor.matmul(gp[:, :], ones_row16[:, ts(j, P)], gate_b16[:, :],
                         start=False, stop=True)
        gsig = sb2.tile([P, HID], F32, tag="gsig")
        nc.scalar.activation(gsig[:, :], gp[:, :], AF.Sigmoid)
        cep = psb.tile([P, HID], F32, tag="b2")
        nc.tensor.matmul(cep[:, :], onehot16[:, ts(j, P)], ct16[:, :],
                         start=True, stop=True)
        dgx = sb2.tile([P, HID], F32, tag="dgx")
        nc.vector.tensor_sub(dgx[:, :], cep[:, :], x_nat[:, j, :])
        nc.vector.tensor_mul(dgx[:, :], gsig[:, :], dgx[:, :])
        nc.vector.tensor_add(x_nat2[:, j, :], x_nat[:, j, :], dgx[:, :])

    xbufs = [x_nat2, x_nat]  # ping-pong

    # ---------------- GAT layers -------------------------------------------
    h_aug16 = sb.tile([P, NT, H * HA + 4], F16)
    nc.vector.memset(h_aug16.rearrange("p a b -> p (a b)"), 1.0)
    xT16 = sb.tile([HID, N], F16)
    hT4 = sb.tile([P, N], F16)  # head h at partitions [h*32, h*32+16)
    st16 = sb.tile([1, H, N], F16)  # s rows per head (partition 0)
    t_cols = sb.tile([P, H * NT], F32)
    s_bc16 = sb.tile([P, H, N], F16)
    rcp4 = sb.tile([1, H, N], F32)
    rcp_bc = sb.tile([HA, H * 512], F32)
    oS32 = sb.tile([HA, NB, H, 512], F32)  # [row, n-half, head, n%512]

    for l in range(3):
        xc = xbufs[l % 2]
        xn_ = xbufs[(l + 1) % 2]
        for j in range(NT):
            pe_T(xc[:, j, :], P, HID, xT16[:, ts(j, P)])
        # hT (head-quad padded): rows h*32+d = h_{head h, dim d}
        hT_ps = psb.tile([P, N], F32, tag="b2")
        for b in range(NB):
            nc.tensor.matmul(hT_ps[:, ts(b, 512)], gat_WTpad16[:, l, :],
                             xT16[:, ts(b, 512)], start=True, stop=True)
        nc.scalar.copy(hT4[:, :], hT_ps[:, :])
        # h natural (aug layout with ones cols preserved)
        for j in range(NT):
            ph = ps.tile([P, P], F16, tag="b1")
            nc.tensor.transpose(ph[:, :], hT4[:, ts(j, P)], ident16)
            nc.vector.tensor_copy(
                bass.AP(tensor=h_aug16.tensor,
                        offset=h_aug16[:, j, 1:].offset,
                        ap=[h_aug16.ap[0], [HA, H], [1, D]]),
                ph[:, :].rearrange("p (h d) -> p h d", h=H)[:, :, 0:D],
            )
        # s rows (out partition h*32), t columns per m-tile
        st_ps = psb.tile([P, N], F32, tag="b2")
        for h in range(H):
            for b in range(NB):
                nc.tensor.matmul(st_ps[h * 32 : h * 32 + 1, ts(b, 512)],
                                 aPad2[h * 32 : h * 32 + D, l, 0:1],
                                 hT4[h * 32 : h * 32 + D, ts(b, 512)],
                                 start=True, stop=True,
                                 tile_position=(h * 32, h * 32))
        for h in range(H):
            nc.vector.tensor_copy(st16[0:1, h, :], st_ps[h * 32 : h * 32 + 1, :])
        tc_ps = ps.tile([P, H * NT], F32, tag="b1")
        for h in range(H):
            for j in range(NT):
                nc.tensor.matmul(tc_ps[:, h * NT + j : h * NT + j + 1],
                                 hT4[h * 32 : h * 32 + D, ts(j, P)],
                                 aPad2[h * 32 : h * 32 + D, l, 1:2],
                                 start=True, stop=True,
                                 tile_position=(h * 32, 0))
        nc.vector.tensor_copy(t_cols[:, :], tc_ps[:, :])
        # s broadcast along partitions
        for h in range(H):
            nc.gpsimd.partition_broadcast(s_bc16[:, h, :], st16[0:1, h, :])

        # e-pipeline over n-halves; each head's accumulation owns one psum bank
        for bh in range(NB):
            oT_ps = ps_o.tile([HA, H * 512], F32, tag="oT")
            for j in range(NT):
                u16 = big.tile([P, H, 512], F16, tag="uy")
                for h in range(H):
                    nc.vector.tensor_scalar(
                        out=u16[:, h, :], in0=s_bc16[:, h, ts(bh, 512)],
                        scalar1=t_cols[:, h * NT + j : h * NT + j + 1],
                        scalar2=None, op0=ALU.add,
                    )
                a16 = big.tile([P, H, 512], F16, tag="aw")
                nc.vector.tensor_scalar(
                    out=a16.rearrange("p a b -> p (a b)"),
                    in0=u16.rearrange("p a b -> p (a b)"),
                    scalar1=0.2, scalar2=None, op0=ALU.mult,
                )
                r16 = big.tile([P, H, 512], F16, tag="r")
                nc.vector.tensor_tensor(
                    out=r16.rearrange("p a b -> p (a b)"),
                    in0=u16.rearrange("p a b -> p (a b)"),
                    in1=a16.rearrange("p a b -> p (a b)"), op=ALU.max,
                )
                y16 = big.tile([P, H, 512], F16, tag="uy")
                nc.scalar.activation(
                    y16.rearrange("p a b -> p (a b)"),
                    r16.rearrange("p a b -> p (a b)"), AF.Exp,
                )
                w16 = big.tile([P, H, 512], F16, tag="aw")
                nc.vector.tensor_tensor(
                    out=w16[:, :, :],
                    in0=y16[:, :, :],
                    in1=rep_ap(maskT16[:, j, ds(bh * 512, 512)], H),
                    op=ALU.mult,
                )
                for h in range(H):
                    nc.tensor.matmul(
                        oT_ps[:, ts(h, 512)],
                        h_aug16[:, j, ds(h * HA, HA)],
                        w16[:, h, :],
                        start=(j == 0), stop=(j == NT - 1),
                    )
            # softmax denominators (row 0 of each head bank) -> scale
            for h in range(H):
                nc.vector.reciprocal_approx_fast(
                    rcp4[0:1, h, ds(bh * 512, 512)], oT_ps[0:1, ts(h, 512)])
            for h in range(H):
                nc.gpsimd.partition_broadcast(
                    rcp_bc[:, ts(h, 512)], rcp4[0:1, h, ds(bh * 512, 512)])
            nc.vector.tensor_mul(
                oS32[:, bh, :, :].rearrange("p a b -> p (a b)"),
                oT_ps[:, :], rcp_bc[:, :])
        # residual + LN back to natural tiles
        for j in range(NT):
            bh, ln = j // (NT // NB), (j % (NT // NB)) * P
            po4 = ps.tile([P, H * HA], F32, tag="b1")
            for h in range(H):
                nc.tensor.transpose(po4[:, ts(h, HA)],
                                    oS32[:, bh, h, ds(ln, P)],
                                    ident32[:HA, :HA])
            z = sb2.tile([P, HID], F32, tag="z")
            nc.vector.tensor_add(
                z[:, :].rearrange("p (h d) -> p h d", h=H),
                bass.AP(tensor=po4.tensor, offset=po4[:, 1:].offset,
                        ap=[po4.ap[0], [HA, H], [1, D]]),
                xc[:, j, :].rearrange("p (h d) -> p h d", h=H))
            layer_norm_tile(z[:, :], xn_[:, j, :], gat_g_bc[l], gat_b_bc[l], False, j)

    xf = xbufs[3 % 2]  # final x after 3 layers -> x_nat  (l=2 wrote xbufs[1])

    # ---------------- phase 3: pooling + output head -----------------------
    xT32 = sb.tile([HID, N], F32)
    for j in range(NT):
        pe_T(xf[:, j, :], P, HID, xT32[:, ts(j, P)], out_via=nc.vector.tensor_copy)
    nc.vector.tensor_copy(xT16[:, :], xT32[:, :])

    mean_col = sb.tile([HID, 1], F32)
    nc.vector.reduce_sum(mean_col[:, :], xT32[:, :], axis=mybir.AxisListType.X)
    g_col = sb.tile([P, 1], F32)
    nc.vector.tensor_scalar(out=g_col[0:HID, :], in0=mean_col[:, :],
                            scalar1=1.0 / N, scalar2=None, op0=ALU.mult)
    nc.vector.reduce_max(g_col[HID:P, :], xT32[:, :], axis=mybir.AxisListType.X)

    sc_ps = psb.tile([1, N], F32, tag="b2")
    for b in range(NB):
        nc.tensor.matmul(sc_ps[:, ts(b, 512)], pool_wT16[:, :], xT16[:, ts(b, 512)],
                         start=True, stop=True)
    tanh_row = sb.tile([1, N], F32)
    nc.scalar.activation(tanh_row[:, :], sc_ps[:, :], AF.Tanh,
                         bias=pool_b_sb[:, :], scale=1.0)
    mx = sb.tile([1, 1], F32)
    nc.vector.reduce_max(mx[:, :], tanh_row[:, :], axis=mybir.AxisListType.X)
    nmx = sb.tile([1, 1], F32)
    nc.vector.tensor_scalar(out=nmx, in0=mx, scalar1=-1.0, scalar2=None, op0=ALU.mult)
    ew_row = sb.tile([1, N], F32)
    zsum = sb.tile([1, 1], F32)
    nc.scalar.activation(ew_row[:, :], tanh_row[:, :], AF.Exp,
                         bias=nmx[:, :], scale=1.0, accum_out=zsum[:, :])
    rcpz = sb.tile([1, 1], F32)
    nc.vector.reciprocal(rcpz[:, :], zsum[:, :])
    rcpz_bc = sb.tile([HID, 1], F32)
    nc.gpsimd.partition_broadcast(rcpz_bc[:, :], rcpz[:, :])

    w_ps = ps.tile([P, NT], F32, tag="b1")
    for j in range(NT):
        nc.tensor.transpose(w_ps[:, j : j + 1], ew_row[:, ts(j, P)], ident32[:1, :1])
    w_cols = sb.tile([P, NT], F32)
    nc.vector.tensor_copy(w_cols, w_ps)
    xa_ps = ps.tile([HID, 1], F32, tag="b1")
    for j in range(NT):
        nc.tensor.matmul(xa_ps[:, :], xf[:, j, :], w_cols[:, j : j + 1],
                         start=(j == 0), stop=(j == NT - 1))
    ga_col = sb.tile([HID, 1], F32)
    nc.vector.tensor_scalar(out=ga_col[:, :], in0=xa_ps[:, :],
                            scalar1=rcpz_bc[:, :], scalar2=None, op0=ALU.mult)

    g16 = sb.tile([P, 1], F16)
    nc.vector.tensor_copy(g16, g_col)
    ga16 = sb.tile([HID, 1], F16)
    nc.vector.tensor_copy(ga16, ga_col)

    ohead_ps = ps.tile([OUT, 1], F32, tag="b1")
    nc.tensor.matmul(ohead_ps[:, :], out_wT16_a[:, :], g16[:, :],
                     start=True, stop=False)
    nc.tensor.matmul(ohead_ps[:, :], out_wT16_b[:, :], ga16[:, :],
                     start=False, stop=False)
    nc.tensor.matmul(ohead_ps[:, :], out_b16[:, :], ones11_16[:, :],
                     start=False, stop=True)
    oc_sb = sb.tile([OUT, 1], F32)
    nc.vector.tensor_copy(oc_sb, ohead_ps)
    orow_ps = ps.tile([1, OUT], F32, tag="b1")
    nc.tensor.transpose(orow_ps[:, :], oc_sb[:, :], ident32)
    orow = sb.tile([1, OUT], F32)
    nc.vector.tensor_copy(orow, orow_ps)

    stats = sb.tile([1, 6], F32)
    mv = sb.tile([1, 2], F32)
    nc.vector.bn_stats(stats[:, :], orow[:, :])
    nc.vector.bn_aggr(mv[:, :], stats[:, :])
    sdo = sb.tile([1, 1], F32)
    nc.scalar.activation(sdo[:, :], mv[:, 1:2], AF.Sqrt, bias=eps_col[0:1, :], scale=1.0)
    rstd = sb.tile([1, 1], F32)
    nc.vector.reciprocal(rstd[:, :], sdo[:, :])
    xn = sb.tile([1, OUT], F32)
    nc.vector.tensor_scalar(out=xn[:, :], in0=orow[:, :], scalar1=mv[:, 0:1],
                            scalar2=rstd[:, :], op0=ALU.subtract, op1=ALU.mult)
    nc.vector.tensor_mul(xn[:, :], xn[:, :], out_ln_g_sb[:, :])
    nc.vector.tensor_add(xn[:, :], xn[:, :], out_ln_b_sb[:, :])
    yrow = sb.tile([1, OUT], F32)
    nc.scalar.activation(yrow[:, :], xn[:, :], AF.Gelu)
    nc.sync.dma_start(out=t["out_y"][:, :], in_=yrow[:, :])

    for pool in (ps_o, psb, ps, big, sb2, sb):
        pool.release()
    _ = padj  # released after phase 1


_CACHE = {}
TRACE = False      # set by test harness to collect an NTFF profile
LAST_RESULT = {}   # test harness reads exec_time_ns / trace path from here


def _get_nc():
    if "nc" not in _CACHE:
        nc = bacc.Bacc()
        build(nc)
        _CACHE["nc"] = nc
    return _CACHE["nc"]


def kernel(**inputs) -> np.ndarray:
    from concourse.bass_utils import run_bass_kernel_spmd

    nc = _get_nc()
    full = {k: np.asarray(v) for k, v in inputs.items()}
    in_maps = []
    for c in range(B):
        m = {
            "adj": np.ascontiguousarray(full["adj"][c], dtype=np.float32),
            "coords": np.ascontiguousarray(full["coords"][c], dtype=np.float32),
            "comm": np.ascontiguousarray(full["comm"][c].reshape(1, N), dtype=np.int32),
            "in_w": np.asarray(full["in_w"], np.float32),
            "in_b": np.asarray(full["in_b"], np.float32).reshape(1, HID),
            "in_ln_g": np.asarray(full["in_ln_g"], np.float32).reshape(1, HID),
            "in_ln_b": np.asarray(full["in_ln_b"], np.float32).reshape(1, HID),
            "comm_table": np.asarray(full["comm_table"], np.float32),
            "gate_w": np.asarray(full["gate_w"], np.float32),
            "gate_b": np.asarray(full["gate_b"], np.float32).reshape(1, HID),
            "gat_W": np.asarray(full["gat_W"], np.float32),
            "gat_a": np.asarray(full["gat_a"], np.float32),
            "gat_ln_g": np.asarray(full["gat_ln_g"], np.float32),
            "gat_ln_b": np.asarray(full["gat_ln_b"], np.float32),
            "pool_w": np.asarray(full["pool_w"], np.float32),
            "pool_b": np.asarray(full["pool_b"], np.float32).reshape(1, 1),
            "out_w": np.asarray(full["out_w"], np.float32),
            "out_b": np.asarray(full["out_b"], np.float32).reshape(1, OUT),
            "out_ln_g": np.asarray(full["out_ln_g"], np.float32).reshape(1, OUT),
            "out_ln_b": np.asarray(full["out_ln_b"], np.float32).reshape(1, OUT),
        }
        in_maps.append(m)
    res = run_bass_kernel_spmd(nc, in_maps, list(range(B)), trace=TRACE)
    LAST_RESULT["res"] = res
    out = np.stack([res.results[c]["out"].reshape(OUT) for c in range(B)], axis=0)
    return out.astype(np.float32)

